# revision 10
# baseline (speedup 1.0000x reference)
"""Trainium2 Bass kernel for the EdgeAttention GNN policy/value network.

Strategy (pure data parallel, 2 samples per core on 8 cores):
  - Per sample, all activations are kept feature-major ("transposed", [D, tokens])
    so every projection is a single PE matmul with the weight as the stationary
    operand and tokens streaming on the free dim.
  - Host permutes each sample's 1024 edge tokens so unmasked tokens come first;
    attention keys are truncated to the first KP=640 slots (binomial(1024,.5)
    never exceeds ~570).  Pad slots are killed with a -1e9 per-partition bias
    folded into the exp() activation.  Queries keep all 1024 tokens except the
    policy head, whose masked outputs are exactly 0 after the final softmax.
  - Per head h, Q^T/K^T rows live at partitions 32h..32h+dh-1, so score matmuls
    target PE row-group h and attention-value matmuls target PE column-group h
    (concurrent 32-wide tiles).  The V matrix gets an extra all-ones column so
    the same accumulating matmul produces the softmax normalizer Z; a one-hot
    gather/scatter matmul pair broadcasts 1/Z back over the head's rows.
  - Softmax uses exp without max subtraction (scores are bounded, |s| < ~10).

Outputs are assembled on host: inverse token permutation, plus the tiny
critic epilogue (critic_vec @ v2_w + v2_b).
"""

import math
import os
import sys

import numpy as np

for _p in ("/opt/trn_rl_repo", "/root/.axon_site/_ro/trn_rl_repo"):
    if os.path.isdir(_p) and _p not in sys.path:
        sys.path.append(_p)

# ---------------------------------------------------------------------------
# Problem constants (hardcoded from the problem spec)
# ---------------------------------------------------------------------------
B, M, F, H = 16, 32, 16, 4
N = M * M  # 1024 edge tokens
NCORES = 8
S = B // NCORES  # samples per core
NEG = -1e9

# (name, Din, dh, Dout, full_queries)
LAYERS = [
    ("d1", 3 * F, 12, 16, True),
    ("d2", 16, 4, 16, True),
    ("pol", 16, 4, 1, False),  # policy head: only unmasked queries matter
    ("val", 16, 4, 1, True),   # value head feeds critic over all tokens
]

DEFAULT_KP = 640  # key slots kept on device (multiple of 128)
_ONES_ROW = np.ones((1, N), np.float32)


def _fr_slices(qn):
    """Free-dim slices of at most 512 (fp32 moving-operand limit)."""
    out = []
    a = 0
    while a < qn:
        out.append((a, min(a + 512, qn)))
        a += 512
    return out


# ---------------------------------------------------------------------------
# Host-side packing
# ---------------------------------------------------------------------------
def _pack_layer_weights(wqkv, bqkv, wo, bo, dh):
    """Build the padded/interleaved device weight layouts for one mha layer."""
    din = wqkv.shape[1]
    dout = wo.shape[1]
    wq = np.zeros((din + 1, 128), np.float32)
    wk = np.zeros((din + 1, 128), np.float32)
    wv = np.zeros((din + 1, 128), np.float32)
    wo_pad = np.zeros((128, dout), np.float32)
    for h in range(H):
        for j in range(dh):
            wq[:din, 32 * h + j] = wqkv[0][:, h * dh + j]
            wq[din, 32 * h + j] = bqkv[0][h * dh + j]
            wk[:din, 32 * h + j] = wqkv[1][:, h * dh + j]
            wk[din, 32 * h + j] = bqkv[1][h * dh + j]
            wv[:din, 32 * h + j] = wqkv[2][:, h * dh + j]
            wv[din, 32 * h + j] = bqkv[2][h * dh + j]
            wo_pad[32 * h + j, :] = wo[h * dh + j, :]
        # ones column -> Z row of the AV matmul (selects the ones row of xT)
        wv[din, 32 * h + dh] = 1.0
    return wq, wk, wv, wo_pad, np.asarray(bo, np.float32).reshape(dout, 1)


def _scatter_mats(dh):
    smat = np.zeros((4, 128), np.float32)
    gmat = np.zeros((128, 4), np.float32)
    for h in range(H):
        for j in range(dh):
            smat[h, 32 * h + j] = 1.0
        gmat[32 * h + dh, h] = 1.0
    return smat, gmat


def _prep_host(inputs, kp):
    """Permute tokens per sample, build fT / bias vectors / packed weights."""
    r1 = np.asarray(inputs["robot_1_node"], np.float32)
    r2 = np.asarray(inputs["robot_2_node"], np.float32)
    edge = np.asarray(inputs["coop_edge_cost"], np.float32).reshape(B, N, F)
    mask = np.asarray(inputs["coop_edge_mask"]).reshape(B, N)

    # f[b, t] = [r1[i], r2[j], edge[ij]] with t = 32 i + j
    f = np.concatenate(
        [
            np.repeat(r1, M, axis=1),
            np.tile(r2, (1, M, 1)),
            edge,
        ],
        axis=2,
    )  # [B, N, 3F]

    perms = np.argsort(mask == 0, axis=1, kind="stable")  # unmasked first
    counts = (mask != 0).sum(axis=1)

    fT = np.ones((B, 3 * F + 1, N), np.float32)
    ebias = np.zeros((B, kp), np.float32)
    for b in range(B):
        fT[b, : 3 * F, :] = f[b, perms[b]].T
        u = int(counts[b])
        if u < kp:
            ebias[b, u:] = NEG

    kpc = kp // 128
    host = {
        "fT": fT.reshape(NCORES, S, 3 * F + 1, N),
        "ebias": ebias.reshape(NCORES, S, kpc, 128, 1),
        "fbias": ebias.reshape(NCORES, S, 1, kp).copy(),
        "perms": perms,
        "counts": counts,
    }

    w = {}
    for nm, _din, dh, _dout, _fq in LAYERS:
        wq, wk, wv, wo_pad, bo = _pack_layer_weights(
            np.asarray(inputs[f"{nm}_wqkv"], np.float32),
            np.asarray(inputs[f"{nm}_bqkv"], np.float32),
            np.asarray(inputs[f"{nm}_wo"], np.float32),
            np.asarray(inputs[f"{nm}_bo"], np.float32),
            dh,
        )
        w[f"wq_{nm}"] = wq
        w[f"wk_{nm}"] = wk
        w[f"wv_{nm}"] = wv
        w[f"wo_{nm}"] = wo_pad
        w[f"bo_{nm}"] = bo
    for dh in (12, 4):
        smat, gmat = _scatter_mats(dh)
        w[f"smat{dh}"] = smat
        w[f"gmat{dh}"] = gmat
    host["weights"] = w
    return host


# ---------------------------------------------------------------------------
# Device module
# ---------------------------------------------------------------------------
def _build_module(kp):
    import concourse.mybir as mybir
    import concourse.tile as tile
    from concourse import bacc

    kpc = kp // 128
    dt = mybir.dt.float32
    AF = mybir.ActivationFunctionType
    AX = mybir.AxisListType

    nc = bacc.Bacc("TRN2", target_bir_lowering=False, debug=False,
                   num_devices=NCORES)

    dtr = mybir.dt.float32r

    fT_d = nc.dram_tensor("fT", [S, 3 * F + 1, N], dt, kind="ExternalInput")
    eb_d = nc.dram_tensor("ebias", [S, kpc, 128, 1], dt, kind="ExternalInput")
    fb_d = nc.dram_tensor("fbias", [S, 1, kp], dt, kind="ExternalInput")
    wd = {}
    for nm, din, dh, dout, _fq in LAYERS:
        wd[f"wq_{nm}"] = nc.dram_tensor(f"wq_{nm}", [din + 1, 128], dt, kind="ExternalInput")
        wd[f"wk_{nm}"] = nc.dram_tensor(f"wk_{nm}", [din + 1, 128], dt, kind="ExternalInput")
        wd[f"wv_{nm}"] = nc.dram_tensor(f"wv_{nm}", [din + 1, 128], dt, kind="ExternalInput")
        wd[f"wo_{nm}"] = nc.dram_tensor(f"wo_{nm}", [128, dout], dt, kind="ExternalInput")
        wd[f"bo_{nm}"] = nc.dram_tensor(f"bo_{nm}", [dout, 1], dt, kind="ExternalInput")
    for dh in (12, 4):
        wd[f"smat{dh}"] = nc.dram_tensor(f"smat{dh}", [4, 128], dt, kind="ExternalInput")
        wd[f"gmat{dh}"] = nc.dram_tensor(f"gmat{dh}", [128, 4], dt, kind="ExternalInput")

    ones_d = nc.dram_tensor("ones_row", [1, N], dt, kind="ExternalInput")
    act_d = nc.dram_tensor("act_out", [S, 1, kp], dt, kind="ExternalOutput")
    cri_d = nc.dram_tensor("cri_out", [S, 1, N], dt, kind="ExternalOutput")

    with tile.TileContext(nc) as tc:
        with (
            tc.tile_pool(name="wp", bufs=1) as wp,
            tc.tile_pool(name="io", bufs=2) as iop,
            tc.tile_pool(name="act", bufs=2 * kpc) as ebp,
            tc.tile_pool(name="qk", bufs=2) as qkp,
            tc.tile_pool(name="va", bufs=2) as vap,
            tc.tile_pool(name="ex", bufs=6) as exp_pool,
            tc.tile_pool(name="so", bufs=2) as sop,
            tc.tile_pool(name="et", bufs=4) as etp,
            tc.tile_pool(name="mi", bufs=2) as mip,
            tc.tile_pool(name="ps", bufs=3, space="PSUM") as psp,
            tc.tile_pool(name="pso", bufs=1, space="PSUM") as psop,
        ):
            # ---- load weights once ----
            wt = {}
            for name, d in wd.items():
                r = name.split("_")[0] in ("wq", "wk", "wv", "wo") or name.startswith("gmat")
                t = wp.tile(list(d.shape), dtr if r else dt, tag=name)
                if r:
                    nc.gpsimd.dma_start(t[:], d.ap())
                else:
                    nc.sync.dma_start(t[:], d.ap())
                wt[name] = t

            for s in range(S):
                fT = iop.tile([3 * F + 1, N], dtr, tag="fT")
                nc.gpsimd.dma_start(fT[:], fT_d[s])
                ebias = []
                for c in range(kpc):
                    t = ebp.tile([128, 1], dt, tag="eb")
                    nc.sync.dma_start(t[:], eb_d[s, c])
                    ebias.append(t)
                fb = iop.tile([1, kp], dt, tag="fb")
                nc.sync.dma_start(fb[:], fb_d[s])

                xT = fT
                outs = {}
                for nm, din, dh, dout, full_q in LAYERS:
                    qn = N if full_q else kp
                    qsl = _fr_slices(qn)
                    scale = 1.0 / math.sqrt(dh)

                    # Q^T / K^T projections (keys only need kp tokens)
                    qtp = psp.tile([128, 1024], dt, tag="ps")
                    for a, b in qsl:
                        nc.tensor.matmul(qtp[:, a:b], lhsT=wt[f"wq_{nm}"][:],
                                         rhs=xT[:, a:b], start=True, stop=True)
                    QTs = qkp.tile([128, 1024], dtr, tag="qt")
                    nc.vector.tensor_copy(QTs[:, 0:qn], qtp[:, 0:qn])

                    ktp = psp.tile([128, 1024], dt, tag="ps")
                    for a, b in _fr_slices(kp):
                        nc.tensor.matmul(ktp[:, a:b], lhsT=wt[f"wk_{nm}"][:],
                                         rhs=xT[:, a:b], start=True, stop=True)
                    KTs = qkp.tile([128, kp], dtr, tag="kt")
                    nc.vector.tensor_copy(KTs[:], ktp[:, 0:kp])

                    # V (token-major) with ones column per head
                    Vaug = vap.tile([128, kpc, 128], mybir.dt.bfloat16, tag="vaug")
                    for c in range(kpc):
                        vp = psp.tile([128, 1024], dt, tag="ps")
                        nc.tensor.matmul(vp[:, 0:128],
                                         lhsT=xT[:, c * 128:(c + 1) * 128],
                                         rhs=wt[f"wv_{nm}"][:],
                                         start=True, stop=True)
                        nc.vector.tensor_copy(Vaug[:, c, :], vp[:, 0:128])

                    # scores -> exp -> AV (accumulating over key chunks)
                    souT = psop.tile([128, 1024], dt, tag="sou")
                    for c in range(kpc):
                        sps, exs = [], []
                        for h in range(H):
                            sp = psp.tile([128, 1024], dt, tag="ps")
                            for a, b in qsl:
                                nc.tensor.matmul(
                                    sp[:, a:b],
                                    lhsT=KTs[32 * h:32 * h + dh,
                                             c * 128:(c + 1) * 128],
                                    rhs=QTs[32 * h:32 * h + dh, a:b],
                                    start=True, stop=True,
                                    tile_position=(32 * h, 0),
                                )
                            sps.append(sp)
                        for h in range(H):
                            ex = exp_pool.tile([128, 1024], mybir.dt.bfloat16, tag="ex")
                            nc.scalar.activation(ex[:, 0:qn], sps[h][:, 0:qn],
                                                 AF.Exp, bias=ebias[c][:],
                                                 scale=scale)
                            exs.append(ex)
                        for a, b in qsl:
                            for h in range(H):
                                nc.tensor.matmul(
                                    souT[32 * h:32 * h + 32, a:b],
                                    lhsT=Vaug[:, c, 32 * h:32 * h + 32],
                                    rhs=exs[h][:, a:b],
                                    start=(c == 0), stop=(c == kpc - 1),
                                    tile_position=(0, 32 * h),
                                    skip_group_check=True,
                                )

                    # normalize: gather Z rows, reciprocal, scatter-broadcast
                    souS = sop.tile([128, 1024], dtr, tag="souS")
                    nc.vector.tensor_copy(souS[:, 0:qn], souT[:, 0:qn])
                    zg = psp.tile([128, 1024], dt, tag="ps")
                    for a, b in qsl:
                        nc.tensor.matmul(zg[0:4, a:b], lhsT=wt[f"gmat{dh}"][:],
                                         rhs=souS[:, a:b], start=True, stop=True)
                    rz = mip.tile([4, 1024], dt, tag="rz")
                    nc.vector.reciprocal_approx_fast(rz[:, 0:qn], zg[0:4, 0:qn])
                    zrt = psp.tile([128, 1024], dt, tag="ps")
                    for a, b in qsl:
                        nc.tensor.matmul(zrt[:, a:b], lhsT=wt[f"smat{dh}"][:],
                                         rhs=rz[:, a:b], start=True, stop=True)
                    soun = sop.tile([128, 1024], dtr, tag="soun")
                    nc.vector.tensor_mul(soun[:, 0:qn], souS[:, 0:qn],
                                         zrt[:, 0:qn])

                    # output projection
                    op = psp.tile([128, 1024], dt, tag="ps")
                    for a, b in qsl:
                        nc.tensor.matmul(op[0:dout, a:b], lhsT=wt[f"wo_{nm}"][:],
                                         rhs=soun[:, a:b], start=True, stop=True)
                    if dout == 1:
                        vec = mip.tile([1, 1024], dt, tag=f"vec_{nm}")
                        nc.vector.tensor_scalar_add(vec[:, 0:qn], op[0:1, 0:qn],
                                                    wt[f"bo_{nm}"][:])
                        outs[nm] = vec
                    else:
                        eT = etp.tile([dout + 1, N], dtr, tag="eT")
                        nc.vector.tensor_scalar_add(eT[0:dout, :], op[0:dout, 0:N],
                                                    wt[f"bo_{nm}"][:])
                        nc.gpsimd.dma_start(eT[dout:dout + 1, :], ones_d.ap())
                        xT = eT

                # final masked softmax over the policy logits (kp slots)
                va2 = mip.tile([1, kp], dt, tag="va2")
                nc.vector.tensor_add(va2[:], outs["pol"][:, 0:kp], fb[:])
                aexp = mip.tile([1, kp], dt, tag="aexp")
                nc.scalar.activation(aexp[:], va2[:], AF.Exp)
                ssum = mip.tile([1, 1], dt, tag="ssum")
                nc.vector.reduce_sum(ssum[:], aexp[:], axis=AX.X)
                rs = mip.tile([1, 1], dt, tag="rs")
                nc.vector.reciprocal(rs[:], ssum[:])
                mact = mip.tile([1, kp], dt, tag="mact")
                nc.vector.tensor_scalar_mul(mact[:], aexp[:], rs[:])
                nc.sync.dma_start(act_d[s], mact[:])
                nc.sync.dma_start(cri_d[s], outs["val"][:, 0:N])

    nc.compile()
    return nc


_MODULE_CACHE = {}


def _get_module(kp):
    if kp not in _MODULE_CACHE:
        _MODULE_CACHE[kp] = _build_module(kp)
    return _MODULE_CACHE[kp]


# ---------------------------------------------------------------------------
# Entry point
# ---------------------------------------------------------------------------
def kernel(**inputs):
    return _run(inputs, trace=False)[:2]


def _install_ntff_hook():
    """Provide antenv.axon_hooks (absent on this image) via ctypes into
    libaxon_pjrt.so so run_bass_kernel_spmd(trace=True) can capture NTFFs."""
    try:
        from antenv.axon_hooks import get_axon_ntff_profile_hook  # noqa: F401
        return
    except ImportError:
        pass
    import contextlib
    import ctypes
    import types

    so_path = "/opt/axon/libaxon_pjrt.so"
    hook = None
    if os.path.exists(so_path):
        lib = ctypes.CDLL(so_path)
        if hasattr(lib, "axon_start_nrt_profile"):
            lib.axon_start_nrt_profile.argtypes = [
                ctypes.POINTER(ctypes.c_int64), ctypes.c_size_t]
            lib.axon_start_nrt_profile.restype = ctypes.c_int64
            lib.axon_stop_nrt_profile.argtypes = [ctypes.c_char_p]
            lib.axon_stop_nrt_profile.restype = ctypes.c_int64

            @contextlib.contextmanager
            def _hook(output_dir, device_ids):
                import jax
                jax.devices()
                if device_ids:
                    ids = (ctypes.c_int64 * len(device_ids))(*device_ids)
                    rc = lib.axon_start_nrt_profile(ids, len(device_ids))
                else:
                    rc = lib.axon_start_nrt_profile(None, 0)
                if rc != 0:
                    raise RuntimeError(f"axon_start_nrt_profile rc={rc}")
                try:
                    yield
                finally:
                    n = lib.axon_stop_nrt_profile(str(output_dir).encode())
                    print(f"ntff profile: {n} file(s) -> {output_dir}",
                          file=sys.stderr)

            hook = _hook

    mod = types.ModuleType("antenv.axon_hooks")
    mod.get_axon_ntff_profile_hook = lambda: hook
    mod.set_axon_ntff_profile_hook = lambda h: None
    sys.modules["antenv.axon_hooks"] = mod


def _run(inputs, trace=False):
    from concourse import bass_utils
    from concourse.bass_utils import run_bass_kernel_spmd

    if trace:
        _install_ntff_hook()
        bass_utils.upload_artifacts = lambda d: d  # no S3 in this container

    mask = np.asarray(inputs["coop_edge_mask"]).reshape(B, N)
    max_cnt = int((mask != 0).sum(axis=1).max())
    kp = DEFAULT_KP
    while kp < max_cnt:
        kp += 128
    kp = min(kp, N)

    host = _prep_host(inputs, kp)
    nc = _get_module(kp)

    in_maps = []
    for c in range(NCORES):
        m = {
            "fT": host["fT"][c],
            "ebias": host["ebias"][c],
            "fbias": host["fbias"][c],
        }
        m.update(host["weights"])
        m["ones_row"] = _ONES_ROW
        in_maps.append(m)

    res = run_bass_kernel_spmd(nc, in_maps, core_ids=list(range(NCORES)),
                               trace=trace)

    act = np.stack([res.results[c]["act_out"] for c in range(NCORES)])
    cri = np.stack([res.results[c]["cri_out"] for c in range(NCORES)])
    act = act.reshape(B, kp)
    cri = cri.reshape(B, N)

    v2_w = np.asarray(inputs["v2_w"], np.float32)
    v2_b = np.asarray(inputs["v2_b"], np.float32)
    perms = host["perms"]

    masked_action = np.zeros((B, N), np.float32)
    critic_vec = np.empty((B, N), np.float32)
    for b in range(B):
        masked_action[b, perms[b][:kp]] = act[b]
        critic_vec[b, perms[b]] = cri[b]
    critic = critic_vec @ v2_w + v2_b
    return masked_action.astype(np.float32), critic.astype(np.float32), res


# ---------------------------------------------------------------------------
# Numpy emulation of the exact device algorithm (for offline validation)
# ---------------------------------------------------------------------------
def _emulate(inputs):
    kp = DEFAULT_KP
    host = _prep_host(inputs, kp)
    w = host["weights"]
    fT_all = host["fT"].reshape(B, 3 * F + 1, N)
    eb_all = host["ebias"].reshape(B, kp)
    act = np.zeros((B, kp), np.float32)
    cri = np.zeros((B, N), np.float32)
    for b in range(B):
        xT = fT_all[b]
        outs = {}
        for nm, din, dh, dout, full_q in LAYERS:
            qn = N if full_q else kp
            scale = np.float32(1.0 / math.sqrt(dh))
            QT = (w[f"wq_{nm}"].T @ xT[:, :qn])  # [128, qn]
            KT = (w[f"wk_{nm}"].T @ xT[:, :kp])  # [128, kp]
            Vaug = xT[:, :kp].T @ w[f"wv_{nm}"]  # [kp, 128]
            souT = np.zeros((128, qn), np.float32)
            for h in range(H):
                sl = slice(32 * h, 32 * h + dh)
                sT = KT[sl, :].T @ QT[sl, :]  # [kp, qn]
                ex = np.exp(scale * sT + eb_all[b][:, None])
                souT[32 * h:32 * h + 32, :] = Vaug[:, 32 * h:32 * h + 32].T @ ex
            z = w[f"gmat{dh}"].T @ souT  # [4, qn]
            rz = (1.0 / z).astype(np.float32)
            zrt = w[f"smat{dh}"].T @ rz  # [128, qn]
            soun = souT * zrt
            op = w[f"wo_{nm}"].T @ soun + w[f"bo_{nm}"]
            if dout == 1:
                outs[nm] = op[0]
            else:
                xT = np.ones((dout + 1, N), np.float32)
                xT[:dout, :] = op
        av = outs["pol"] + eb_all[b]
        aexp = np.exp(av)
        act[b] = aexp / aexp.sum()
        cri[b] = outs["val"]

    v2_w = np.asarray(inputs["v2_w"], np.float32)
    v2_b = np.asarray(inputs["v2_b"], np.float32)
    perms = host["perms"]
    masked_action = np.zeros((B, N), np.float32)
    critic_vec = np.empty((B, N), np.float32)
    for b in range(B):
        masked_action[b, perms[b][:kp]] = act[b]
        critic_vec[b, perms[b]] = cri[b]
    critic = critic_vec @ v2_w + v2_b
    return masked_action.astype(np.float32), critic.astype(np.float32)
